# revision 9
# baseline (speedup 1.0000x reference)
"""Causal attention (with faithful missing-head-transpose reshape bug) on 8 Trainium2 cores.

Problem: B=2, T=2048, E=1024, H=16, dk=64.
  qkv = x @ w_qkv.T ; q,k,v split; per-head causal softmax attention;
  out = att_out[B,H,T,dk].reshape(B,T,E)  (NO head transpose — faithful bug);
  y = out @ w_proj.T + b_proj

Key observation: because of the missing transpose, output rows
y[b, 128h : 128h+128, :] depend ONLY on head h.  Sharding (batch x head-group)
over 8 cores therefore needs NO collectives: core c handles batch c//4 and
heads 4*(c%4) .. 4*(c%4)+3, producing output rows [512g, 512g+512) of batch b.

Per-core kernel (bf16 matmuls, fp32 PSUM accumulation, fp32 softmax math):
  - host supplies x^T [E,T] and pre-transposed weight slices in bf16
  - Q^T,K^T [256,T] and V [T,256] via matmuls
  - scores computed TRANSPOSED: S^T[j,i] (keys on partitions) so that
    exp(S^T) is directly the lhsT-ready P^T for the P@V matmul
  - V is stored with a ones-column per head: the P@V matmul's row 64 is then
    the softmax denominator for free
  - causal masking via gpsimd.affine_select on the exp tiles (diag blocks only;
    fully-masked blocks are skipped entirely)
  - normalization: fast-approx reciprocal of the denom row, broadcast across
    partitions with a K=1 outer-product matmul, one DVE multiply
  - the buggy reshape is free in row-major DRAM; the projection's lhsT
    (R^T chunks) is read from a [dup, shift-by-one] att2 buffer with a
    stride-16 access pattern
"""

import os
import sys

import numpy as np

for _p in ("/opt/trn_rl_repo", "/root/.axon_site/_ro/trn_rl_repo"):
    if os.path.isdir(_p) and _p not in sys.path:
        sys.path.insert(0, _p)

import ml_dtypes  # noqa: E402

import concourse.bacc as bacc  # noqa: E402
import concourse.mybir as mybir  # noqa: E402
from concourse.bass import ds, ts  # noqa: E402
from concourse.tile import TileContext  # noqa: E402

F32 = mybir.dt.float32
BF16 = mybir.dt.bfloat16
AF = mybir.ActivationFunctionType
BF16NP = ml_dtypes.bfloat16

P = 128
E = 1024
DK = 64
HPC = 4  # heads per core
TW = 512  # i-window for scores / pv matmuls
EC = E // P  # 8 e-chunks
DC = (HPC * DK) // P  # 2 chunks of per-core qk features
FW = E // 512  # 2 output-feature windows


def build_nc(T=2048):
    W = T // TW  # i-windows
    JPW = TW // P  # j-chunks per window (4)
    TC = T // P  # t-chunks for V
    RR = (T * DK) // E  # rows of R per head (T/16)
    TT = E // DK  # 16 t-positions per R row

    nc = bacc.Bacc("TRN2", target_bir_lowering=False, debug=False)
    xT = nc.declare_dram_parameter("xT", [E, T], BF16, isOutput=False)
    wqT = nc.declare_dram_parameter("wqT", [E, HPC * DK], BF16, isOutput=False)
    wkT = nc.declare_dram_parameter("wkT", [E, HPC * DK], BF16, isOutput=False)
    wvT = nc.declare_dram_parameter("wvT", [E, HPC * DK], BF16, isOutput=False)
    wpT = nc.declare_dram_parameter("wpT", [E, E], BF16, isOutput=False)
    bp = nc.declare_dram_parameter("bp", [1, E], BF16, isOutput=False)
    y = nc.declare_dram_parameter("y", [HPC * RR, E], F32, isOutput=True)

    with nc.allow_low_precision(reason="bf16 matmuls; accumulation stays fp32 in PSUM"), TileContext(nc) as tc:
        with (
            tc.tile_pool(name="const", bufs=1) as const,
            tc.tile_pool(name="qkvout", bufs=1) as qkv_pool,
            tc.tile_pool(name="wp", bufs=1) as wp_pool,
        ):
            ones = const.tile([P, P], BF16)
            nc.vector.memset(ones, 1.0)
            bp_sb = const.tile([1, E], BF16)
            nc.sync.dma_start(out=bp_sb, in_=bp[:, :])

            wp_sb = wp_pool.tile([P, EC, E], BF16)

            qT = qkv_pool.tile([P, DC, T], BF16)
            kT = qkv_pool.tile([P, DC, T], BF16)
            vsb = qkv_pool.tile([P, TC, HPC * (DK + 1)], BF16)
            # ones column per head (row 64 of each P@V psum = softmax denominator)
            nc.vector.memset(
                vsb.rearrange("p t (h c) -> p t h c", c=DK + 1)[:, :, :, DK : DK + 1], 1.0
            )
            zer = const.tile([P, P], BF16)
            nc.vector.memset(zer, 0.0)
            wsrc = const.tile([P, TW], BF16)
            nc.vector.memset(wsrc, 0.0)
            # single lower-triangle-inclusive mask (keep j_local <= i_local);
            # diagonal blocks apply it at column offset 128*qq, and the fully
            # masked region left of it is never computed at all (N-trim)
            trimask = const.tile([P, P], BF16, name="trimask", tag="trimask")
            nc.vector.memset(trimask, 1.0)
            nc.gpsimd.affine_select(
                out=trimask,
                in_=trimask,
                pattern=[[1, P]],
                compare_op=mybir.AluOpType.is_ge,
                fill=0.0,
                base=0,
                channel_multiplier=-1,
            )

            # ---------------- QKV projections ----------------
            with (
                tc.tile_pool(name="xin", bufs=1) as xpool,
                tc.tile_pool(name="wqkv", bufs=1) as wq_pool,
                tc.tile_pool(name="psq", bufs=4, space="PSUM") as psq,
            ):
                wq_sb = wq_pool.tile([P, EC, HPC * DK], BF16)
                wk_sb = wq_pool.tile([P, EC, HPC * DK], BF16)
                wv_sb = wq_pool.tile([P, EC, HPC * DK], BF16)
                # spread the input load across several engines' DMA queues;
                # priority order: wq then x (gates the first QK matmuls),
                # then wk/wv, then w_proj (needed only at the projection)
                engs = [nc.sync, nc.gpsimd, nc.scalar]
                xp = xpool.tile([P, EC, T], BF16)
                for e in range(EC):
                    engs[e % 3].dma_start(out=wq_sb[:, e, :], in_=wqT[ts(e, P), :])
                for e in range(EC):
                    engs[e % 3].dma_start(out=xp[:, e, :], in_=xT[ts(e, P), :])
                for e in range(EC):
                    engs[(e + 1) % 3].dma_start(out=wk_sb[:, e, :], in_=wkT[ts(e, P), :])
                    engs[(e + 2) % 3].dma_start(out=wv_sb[:, e, :], in_=wvT[ts(e, P), :])
                # w_proj is only needed at the projection stage — load it last
                for e in range(EC):
                    engs[e % 3].dma_start(
                        out=wp_sb[:, e, :], in_=wpT[ts(e, P), :]
                    )


                # Q^T, K^T : [dloc (part), T]
                for dst, wsb in ((qT, wq_sb), (kT, wk_sb)):
                    for dc in range(DC):
                        for w in range(T // TW):
                            ps = psq.tile([P, TW], F32, tag="qa", name="ps_qk")
                            for e in range(EC):
                                nc.tensor.matmul(
                                    ps,
                                    wsb[:, e, ts(dc, P)],
                                    xp[:, e, ds(TW * w, TW)],
                                    start=(e == 0),
                                    stop=(e == EC - 1),
                                )
                            nc.vector.tensor_copy(dst[:, dc, ds(TW * w, TW)], ps)

                # V natural : [t (part), d]
                for t in range(TC):
                    psv = psq.tile([P, HPC * DK], F32, tag="qa", name="ps_v")
                    for e in range(EC):
                        nc.tensor.matmul(
                            psv,
                            xp[:, e, ts(t, P)],
                            wv_sb[:, e, :],
                            start=(e == 0),
                            stop=(e == EC - 1),
                        )
                    nc.vector.tensor_copy(
                        vsb.rearrange("p t (h c) -> p t h c", c=DK + 1)[:, t, :, 0:DK],
                        psv.rearrange("p (h d) -> p h d", d=DK),
                    )

            # ---------------- attention ----------------
            with tc.tile_pool(name="att", bufs=1) as att_pool:
                att2 = []
                for h in range(HPC):
                    a = att_pool.tile([P, T], BF16, name=f"att2_{h}", tag=f"att2_{h}")
                    att2.append(a)
                    # last col of shifted half never written; keep sim happy
                    nc.vector.memset(a[DK : 2 * DK, T - 1 : T], 0.0)

                with (
                    tc.tile_pool(name="exps", bufs=8) as epool,
                    tc.tile_pool(name="rec", bufs=2) as rpool,
                    tc.tile_pool(name="psa", bufs=1, space="PSUM") as psa,
                ):
                    WARMERS = 3  # zero-matmuls per jc to keep the PE HAM-warm
                    NSP = HPC * TW // P  # denom elems per lane after spread
                    norm2 = None
                    for w in range(W):
                        pvs = [
                            psa.tile([P, TW], F32, tag=f"pv{h}", bufs=1, name=f"pv{h}")
                            for h in range(HPC)
                        ]
                        njc = JPW * (w + 1)
                        pend = []
                        for jc in range(njc):
                            qq = jc - JPW * w  # >=0 on causal-diagonal blocks
                            t0 = P * qq if qq > 0 else 0  # fully-masked cols skipped
                            ess = []
                            for p in range(2):
                                st = psa.tile([P, 2 * TW], F32, tag="s", bufs=2, name="st")
                                for sub in range(2):
                                    nc.tensor.matmul(
                                        st[:, ds(TW * sub + t0, TW - t0)],
                                        kT[ds(DK * sub, DK), p, ts(jc, P)],
                                        qT[ds(DK * sub, DK), p, ds(TW * w + t0, TW - t0)],
                                        start=True,
                                        stop=True,
                                    )
                                es = epool.tile([P, 2 * TW], BF16, name="es")
                                if t0 == 0:
                                    nc.scalar.activation(es, st, AF.Exp, scale=1.0 / 8.0)
                                else:
                                    nc.scalar.activation(
                                        es.rearrange("p (s n) -> p s n", s=2)[:, :, t0:TW],
                                        st.rearrange("p (s n) -> p s n", s=2)[:, :, t0:TW],
                                        AF.Exp,
                                        scale=1.0 / 8.0,
                                    )
                                if qq >= 0:
                                    for sub in range(2):
                                        nc.vector.tensor_mul(
                                            es[:, ds(TW * sub + t0, P)],
                                            es[:, ds(TW * sub + t0, P)],
                                            trimask,
                                        )
                                ess.append(es)
                            pend.append((ess, jc, t0))
                            if len(pend) > 2:
                                e0 = pend.pop(0)
                                _emit_pv(nc, pvs, vsb, zer, wsrc, e0[0], e0[1], njc, WARMERS, e0[2])
                            if jc == 3 and norm2 is not None:
                                norm2()
                                norm2 = None
                        for e0 in pend:
                            _emit_pv(nc, pvs, vsb, zer, wsrc, e0[0], e0[1], njc, 0, e0[2])

                        # ---- normalization part 1: drain pv banks ----
                        dns = rpool.tile([P, HPC * TW], F32, name="dns", tag="dns")
                        praws = []
                        for h in range(HPC):
                            praw = rpool.tile([P, TW], BF16, name="praw", tag=f"praw{h}", bufs=2)
                            nc.vector.tensor_copy(praw[0:DK, :], pvs[h][0:DK, :])
                            nc.vector.tensor_copy(
                                dns[DK : DK + 1, ds(TW * h, TW)], pvs[h][DK : DK + 1, :]
                            )
                            praws.append(praw)

                        def _norm2(w=w, dns=dns, praws=praws, proj=None):
                            # 1-lane fast-approx reciprocal of the denom row, cast
                            # to bf16 in place; the rt matmul broadcasts the row
                            # across 64 partitions (no DMA spread round-trip)
                            rec32 = rpool.tile([P, HPC * TW], F32, name="rec32", tag="rec32")
                            nc.vector.reciprocal(
                                out=rec32[DK : DK + 1, :], in_=dns[DK : DK + 1, :]
                            )
                            recb = rpool.tile([P, HPC * TW], BF16, name="recb", tag="recb")
                            nc.vector.tensor_copy(
                                recb[DK : DK + 1, :], rec32[DK : DK + 1, :]
                            )
                            for h in range(HPC):
                                rt = psa.tile([P, 2 * TW], F32, tag="s", bufs=2, name="rt")
                                nc.tensor.matmul(
                                    rt[0:DK, 0:TW],
                                    ones[DK : DK + 1, 0:DK],
                                    recb[DK : DK + 1, ds(TW * h, TW)],
                                    start=True,
                                    stop=True,
                                )
                                nc.vector.tensor_mul(
                                    att2[h][0:DK, ds(TW * w, TW)],
                                    rt[0:DK, 0:TW],
                                    praws[h][0:DK, :],
                                )
                                if w == 0:
                                    nc.sync.dma_start(
                                        out=att2[h][DK : 2 * DK, 0 : TW - 1],
                                        in_=att2[h][0:DK, 1:TW],
                                    )
                                else:
                                    nc.sync.dma_start(
                                        out=att2[h][DK : 2 * DK, TW * w - 1 : TW * (w + 1) - 1],
                                        in_=att2[h][0:DK, ds(TW * w, TW)],
                                    )
                                if proj is not None:
                                    proj(h)

                        norm2 = _norm2

                    # cover the last window's normalization-chain latency with a
                    # PE warm chain, then run it with the per-head projection fused in
                    with tc.tile_pool(name="yout", bufs=2) as ypool:

                        def _proj(h):
                            a2v = att2[h].rearrange("p (r t) -> p r t", t=TT)
                            for fw in range(FW):
                                yp = psa.tile([P, TW], F32, tag=f"pv{h}", bufs=1, name="yp")
                                for m in range(EC):
                                    nc.tensor.matmul(
                                        yp[0:RR, :],
                                        a2v[:, :, 2 * m : 2 * m + 1],
                                        wp_sb[:, m, ds(512 * fw, 512)],
                                        start=(m == 0),
                                        stop=False,
                                    )
                                nc.tensor.matmul(
                                    yp[0:RR, :],
                                    ones[0:1, 0:RR],
                                    bp_sb[0:1, ds(512 * fw, 512)],
                                    start=False,
                                    stop=True,
                                )
                                ysb = ypool.tile([P, 512], F32, name="ysb")
                                nc.vector.tensor_copy(ysb[0:RR, :], yp[0:RR, :])
                                nc.sync.dma_start(
                                    out=y[ds(RR * h, RR), ds(512 * fw, 512)], in_=ysb[0:RR, :]
                                )

                        wtf = psa.tile([P, 2 * TW], F32, tag="s", bufs=2, name="wtf")
                        NWARMF = 30
                        for i in range(NWARMF):
                            nc.tensor.matmul(
                                wtf[0 : DK + 1, 0:TW],
                                zer[:, 0 : DK + 1],
                                wsrc,
                                start=(i == 0),
                                stop=(i == NWARMF - 1),
                            )
                        norm2(proj=_proj)
    nc.compile()
    return nc


def _emit_pv(nc, pvs, vsb, zer, wsrc, ess, jc, njc, warmers, t0=0):
    """P@V matmuls (M=65: V plus ones column -> denominator row) for one jc,
    then `warmers` zero-matmuls that accumulate 0 into an open pv group —
    pure PE-activity filler so the HAM clock gate stays at full speed.
    On diagonal blocks (t0>0) the fully-masked leading columns are skipped."""
    for p in range(2):
        es = ess[p]
        for sub in range(2):
            h = 2 * p + sub
            nc.tensor.matmul(
                pvs[h][0 : DK + 1, ds(t0, TW - t0)],
                vsb[:, jc, ds((DK + 1) * h, DK + 1)],
                es[:, ds(TW * sub + t0, TW - t0)],
                start=(jc == 0),
                stop=(jc == njc - 1),
                skip_group_check=(t0 > 0),
            )
    if jc < njc - 1:
        for k in range(warmers):
            nc.tensor.matmul(
                pvs[(jc + k) % HPC][0 : DK + 1, :],
                zer[:, 0 : DK + 1],
                wsrc,
                start=False,
                stop=False,
            )


_CACHE = {}
LAST_RESULT = None


def _get_nc(T=2048):
    key = ("nc", T)
    if key not in _CACHE:
        _CACHE[key] = build_nc(T=T)
    return _CACHE[key]


def make_in_maps(x, w_qkv, w_proj, b_proj):
    B, T, _E = x.shape
    in_maps = []
    wpTh = np.ascontiguousarray(w_proj.T.astype(BF16NP))
    bph = np.ascontiguousarray(b_proj.reshape(1, E).astype(BF16NP))
    xTs = [np.ascontiguousarray(x[b].T.astype(BF16NP)) for b in range(B)]
    for c in range(8):
        b, g = divmod(c, 4)
        r0 = HPC * DK * g  # 256*g
        sl = slice(r0, r0 + HPC * DK)
        in_maps.append(
            {
                "xT": xTs[b],
                "wqT": np.ascontiguousarray(w_qkv[sl, :].T.astype(BF16NP)),
                "wkT": np.ascontiguousarray(w_qkv[E:][sl, :].T.astype(BF16NP)),
                "wvT": np.ascontiguousarray(w_qkv[2 * E :][sl, :].T.astype(BF16NP)),
                "wpT": wpTh,
                "bp": bph,
            }
        )
    return in_maps


def kernel(x, w_qkv, w_proj, b_proj):
    global LAST_RESULT
    from concourse.bass_utils import run_bass_kernel_spmd

    x = np.asarray(x, dtype=np.float32)
    w_qkv = np.asarray(w_qkv, dtype=np.float32)
    w_proj = np.asarray(w_proj, dtype=np.float32)
    b_proj = np.asarray(b_proj, dtype=np.float32)
    B, T, _E = x.shape

    nc = _get_nc(T=T)
    in_maps = make_in_maps(x, w_qkv, w_proj, b_proj)
    res = run_bass_kernel_spmd(nc, in_maps, core_ids=list(range(8)))
    LAST_RESULT = res

    out = np.empty((B, T, E), dtype=np.float32)
    rows = HPC * ((T * DK) // E)  # 512 rows per core
    for c in range(8):
        b, g = divmod(c, 4)
        out[b, rows * g : rows * (g + 1), :] = res.results[c]["y"]
    return out



# revision 10
# speedup vs baseline: 1.1501x; 1.1501x over previous
"""Causal attention (with faithful missing-head-transpose reshape bug) on 8 Trainium2 cores.

Problem: B=2, T=2048, E=1024, H=16, dk=64.
  qkv = x @ w_qkv.T ; q,k,v split; per-head causal softmax attention;
  out = att_out[B,H,T,dk].reshape(B,T,E)  (NO head transpose — faithful bug);
  y = out @ w_proj.T + b_proj

Because of the missing transpose, output rows y[b, 128h:128h+128, :] depend
ONLY on head h, so (batch x head-group) sharding over 8 cores needs no
collectives: core c handles batch c//4 and heads 4*(c%4)..4*(c%4)+3.

v2 schedule (single fully-interleaved pipeline; PE is the bottleneck engine
at ~130us of useful work, everything else hides under it):
  - input DMA is window-sliced and priority-ordered (wq, x-w0, wk, wv, x-w1,
    x-w2, wp, x-w3) so the first QKV matmuls start ~2us in; a PE warm chain
    runs during the DMA latency so the clock p-state is ramped before real work
  - QKV projection for window w+1 is emitted as fill closures pulled between
    attention jc-groups of window w — real work replaces v1's zero-matmul
    "HAM warmers"
  - scores are computed transposed (keys on partitions); exp tiles double as
    lhsT-ready P^T; diagonal blocks skip their fully-masked leading columns
    (N-trim) in scores/exp/PV and apply a single 128x128 triangle mask
  - V tiles are 128 wide per head (64 V cols, a ones column at 64+h, zeros
    elsewhere) so each head's softmax denominator lands on its own psum
    partition; the reciprocal then runs on [4,512] DVE lanes — no DMA spread
    round-trip and no 1-lane ops (DVE reciprocal is ~6.3 cycles/element)
  - the output projection for window w runs DURING window w+1 with all four
    heads stacked into M=128 (att2a is h-major so one lhsT AP spans heads),
    reusing freed pv PSUM banks; only window 3's norm+proj remain as tail
"""

import os
import sys
from collections import deque

import numpy as np

for _p in ("/opt/trn_rl_repo", "/root/.axon_site/_ro/trn_rl_repo"):
    if os.path.isdir(_p) and _p not in sys.path:
        sys.path.insert(0, _p)

import ml_dtypes  # noqa: E402

import concourse.bacc as bacc  # noqa: E402
import concourse.mybir as mybir  # noqa: E402
from concourse.bass import ds, ts  # noqa: E402
from concourse.tile import TileContext  # noqa: E402

F32 = mybir.dt.float32
BF16 = mybir.dt.bfloat16
AF = mybir.ActivationFunctionType
BF16NP = ml_dtypes.bfloat16

P = 128
E = 1024
DK = 64
HPC = 4  # heads per core
TW = 512  # i-window for scores / pv matmuls
EC = E // P  # 8 e-chunks
DC = (HPC * DK) // P  # 2 chunks of per-core qk features
FW = E // 512  # 2 output-feature windows


def build_nc(T=2048):
    W = T // TW  # 4 i-windows
    JPW = TW // P  # 4 j-chunks per window
    TC = T // P  # 16 t-chunks for V
    RR = (T * DK) // E  # 128 rows of R per head
    TT = E // DK  # 16 t-positions per R row
    RPW = RR // W  # 32 rows per head per window

    nc = bacc.Bacc("TRN2", target_bir_lowering=False, debug=False)
    xT = nc.declare_dram_parameter("xT", [E, T], BF16, isOutput=False)
    # q|k|v weight slices concatenated on the host: 3x fewer, 3x larger DMA
    # descriptors than three separate 512B-per-row loads
    wqkvT = nc.declare_dram_parameter("wqkvT", [E, 3 * HPC * DK], BF16, isOutput=False)
    wpT = nc.declare_dram_parameter("wpT", [E, E], BF16, isOutput=False)
    bp = nc.declare_dram_parameter("bp", [1, E], BF16, isOutput=False)
    y = nc.declare_dram_parameter("y", [HPC * RR, E], F32, isOutput=True)

    with nc.allow_low_precision(reason="bf16 matmuls; accumulation stays fp32 in PSUM"), TileContext(nc) as tc:
        with (
            tc.tile_pool(name="const", bufs=1) as const,
            tc.tile_pool(name="wts", bufs=1) as wts,
            tc.tile_pool(name="xin", bufs=1) as xpool,
            tc.tile_pool(name="qkvout", bufs=1) as qkv_pool,
            tc.tile_pool(name="att", bufs=1) as att_pool,
            tc.tile_pool(name="exps", bufs=15) as epool,
            tc.tile_pool(name="rec", bufs=2) as rpool,
            tc.tile_pool(name="yout", bufs=2) as ypool,
            tc.tile_pool(name="psa", bufs=1, space="PSUM") as psa,
        ):
            # ---------------- constants ----------------
            ones = const.tile([P, P], BF16)
            nc.vector.memset(ones, 1.0)
            zer = const.tile([P, P], BF16)
            nc.vector.memset(zer, 0.0)
            wsrc = const.tile([P, TW], BF16)
            nc.vector.memset(wsrc, 0.0)
            # lower-triangle-inclusive mask (keep i_local >= j_local)
            trimask = const.tile([P, P], BF16, name="trimask", tag="trimask")
            nc.vector.memset(trimask, 1.0)
            nc.gpsimd.affine_select(
                out=trimask,
                in_=trimask,
                pattern=[[1, P]],
                compare_op=mybir.AluOpType.is_ge,
                fill=0.0,
                base=0,
                channel_multiplier=-1,
            )
            bp_sb = const.tile([1, E], BF16)
            # head-selector for the reciprocal broadcast matmul: row j of
            # column block h is 1 iff j==h (K=4 matmul, base partition 0);
            # built with two affine_selects since memsets can't start at
            # partitions other than 0/32/64
            sel = const.tile([P, HPC * DK], BF16, name="sel", tag="sel")
            nc.vector.memset(sel[0:HPC, :], 1.0)
            nc.gpsimd.affine_select(
                out=sel[0:HPC, :],
                in_=sel[0:HPC, :],
                pattern=[[1, HPC * DK]],
                compare_op=mybir.AluOpType.is_ge,
                fill=0.0,
                base=0,
                channel_multiplier=-DK,
            )
            nc.gpsimd.affine_select(
                out=sel[0:HPC, :],
                in_=sel[0:HPC, :],
                pattern=[[-1, HPC * DK]],
                compare_op=mybir.AluOpType.is_ge,
                fill=0.0,
                base=DK - 1,
                channel_multiplier=DK,
            )

            wqkv_sb = wts.tile([P, EC, 3 * HPC * DK], BF16)
            wp_sb = wts.tile([P, EC, E], BF16)
            xp = xpool.tile([P, EC, T], BF16)

            qT = qkv_pool.tile([P, DC, T], BF16)
            kT = qkv_pool.tile([P, DC, T], BF16)
            vsb = qkv_pool.tile([P, TC, HPC * (DK + 1)], BF16)
            v4 = vsb.rearrange("p t (h c) -> p t h c", c=DK + 1)
            # ones column per head (PV psum row 64 = softmax denominator)
            nc.vector.memset(v4[:, :, :, DK : DK + 1], 1.0)

            # attention output, (r, h, t)-interleaved columns: col r*64+h*16+t;
            # rows 0:64 hold att[t], rows 64:128 hold att[t+1] (shifted copy;
            # only even-t columns of the shifted half are ever read). With this
            # order the projection's stacked-M index m = r'*4+h is a SINGLE
            # stride-16 AP dimension (BIR allows only one free dim on the
            # stationary matmul operand).
            att2a = att_pool.tile([P, HPC * T], BF16, name="att2a", tag="att2a")
            aI = att2a.rearrange("p (r h t) -> p r h t", h=HPC, t=TT)
            vM = att2a.rearrange("p (m s) -> p m s", s=TT)

            # dummy exp: pulls the ACT exp table load into the DMA wait
            es_d = epool.tile([P, 2 * TW], BF16, name="es_d")
            nc.scalar.activation(es_d[:, 0:8], zer[:, 0:8], AF.Exp, scale=1.0)

            # ---------------- input DMA schedule ----------------
            engs = [nc.sync, nc.gpsimd, nc.scalar]
            for e in range(EC):
                engs[e % 3].dma_start(out=wqkv_sb[:, e, :], in_=wqkvT[ts(e, P), :])
            for e in range(EC):
                engs[e % 3].dma_start(
                    out=xp[:, e, ds(0, TW)], in_=xT[ts(e, P), ds(0, TW)]
                )
            engs[0].dma_start(out=bp_sb, in_=bp[:, :])
            for wd in (1, 2):
                for e in range(EC):
                    engs[e % 3].dma_start(
                        out=xp[:, e, ds(TW * wd, TW)], in_=xT[ts(e, P), ds(TW * wd, TW)]
                    )
            for e in range(EC):
                engs[e % 3].dma_start(out=wp_sb[:, e, :], in_=wpT[ts(e, P), :])
            for e in range(EC):
                engs[e % 3].dma_start(
                    out=xp[:, e, ds(TW * 3, TW)], in_=xT[ts(e, P), ds(TW * 3, TW)]
                )

            # ---------------- PE warm chain during the DMA wait ----------------
            warm = psa.tile([P, 2 * TW], F32, tag="s", bufs=2, name="warm")
            NWARM0 = 10
            for i in range(NWARM0):
                nc.tensor.matmul(
                    warm[0 : DK + 1, 0:TW],
                    zer[:, 0 : DK + 1],
                    wsrc,
                    start=(i == 0),
                    stop=False,
                )

            # ---------------- QKV emission (window 0 eager, rest as fills) ----
            def mk_qk(dst, coff, wd, dc, warmers=0):
                def go():
                    ps = psa.tile([P, 2 * TW], F32, tag="s", bufs=2, name="ps_qk")
                    for e in range(EC):
                        nc.tensor.matmul(
                            ps[:, 0:TW],
                            wqkv_sb[:, e, ds(coff + P * dc, P)],
                            xp[:, e, ds(TW * wd, TW)],
                            start=(e == 0),
                            stop=(e == EC - 1),
                        )
                        for k in range(warmers):
                            nc.tensor.matmul(
                                warm[0 : DK + 1, 0:TW],
                                zer[:, 0 : DK + 1],
                                wsrc,
                                start=False,
                                stop=(e == EC - 1 and k == warmers - 1),
                            )
                    nc.vector.tensor_copy(dst[:, dc, ds(TW * wd, TW)], ps[:, 0:TW])

                return go

            def mk_v(t):
                def go():
                    ps = psa.tile([P, 2 * TW], F32, tag="s", bufs=2, name="ps_v")
                    for e in range(EC):
                        nc.tensor.matmul(
                            ps[:, 0 : HPC * DK],
                            xp[:, e, ts(t, P)],
                            wqkv_sb[:, e, ds(2 * HPC * DK, HPC * DK)],
                            start=(e == 0),
                            stop=(e == EC - 1),
                        )
                    nc.vector.tensor_copy(
                        v4[:, t, :, 0:DK],
                        ps[:, 0 : HPC * DK].rearrange("p (h d) -> p h d", d=DK),
                    )

                return go

            def qkv_fills(wd):
                out = [mk_qk(kT, HPC * DK, wd, dc) for dc in range(DC)]
                out += [mk_qk(qT, 0, wd, dc) for dc in range(DC)]
                out += [mk_v(t) for t in range(JPW * wd, JPW * (wd + 1))]
                return out

            # window 0, DMA-paced; warm matmuls interleave with the first chain
            mk_qk(qT, 0, 0, 0, warmers=1)()
            mk_qk(qT, 0, 0, 1)()
            mk_qk(kT, HPC * DK, 0, 0)()
            mk_qk(kT, HPC * DK, 0, 1)()
            for t in range(JPW):
                mk_v(t)()

            fills = deque(qkv_fills(1))

            # ---------------- attention + fused projection ----------------
            defw = deque()  # deferred norm/proj closures for window w-1
            for w in range(W):
                pvs_box = [None]

                def get_pvs(pvs_box=pvs_box):
                    if pvs_box[0] is None:
                        pvs_box[0] = [
                            psa.tile([P, TW], F32, tag=f"pv{h}", bufs=1, name=f"pv{h}")
                            for h in range(HPC)
                        ]
                    return pvs_box[0]

                njc = JPW * (w + 1)
                pend = []
                pop_from = 5 if w > 0 else 2
                for jc in range(njc):
                    qq = jc - JPW * w  # >=0 on causal-diagonal blocks
                    t0 = P * qq if qq > 0 else 0
                    ess = []
                    for p in range(2):
                        st = psa.tile([P, 2 * TW], F32, tag="s", bufs=2, name="st")
                        for sub in range(2):
                            nc.tensor.matmul(
                                st[:, ds(TW * sub + t0, TW - t0)],
                                kT[ds(DK * sub, DK), p, ts(jc, P)],
                                qT[ds(DK * sub, DK), p, ds(TW * w + t0, TW - t0)],
                                start=True,
                                stop=True,
                            )
                        es = epool.tile([P, 2 * TW], BF16, name="es")
                        if t0 == 0:
                            nc.scalar.activation(es, st, AF.Exp, scale=1.0 / 8.0)
                        else:
                            nc.scalar.activation(
                                es.rearrange("p (s n) -> p s n", s=2)[:, :, t0:TW],
                                st.rearrange("p (s n) -> p s n", s=2)[:, :, t0:TW],
                                AF.Exp,
                                scale=1.0 / 8.0,
                            )
                        if qq >= 0:
                            for sub in range(2):
                                nc.vector.tensor_mul(
                                    es[:, ds(TW * sub + t0, P)],
                                    es[:, ds(TW * sub + t0, P)],
                                    trimask,
                                )
                        ess.append(es)
                    pend.append((ess, jc, t0))

                    if jc in (2, 3, 4) and defw:
                        defw.popleft()()
                    if len(pend) > 2 and jc >= pop_from:
                        e0 = pend.pop(0)
                        _emit_pv(nc, get_pvs(), vsb, e0[0], e0[1], njc, e0[2])
                    for _ in range(2 if jc < 4 else 1):
                        if fills:
                            fills.popleft()()
                for e0 in pend:
                    _emit_pv(nc, get_pvs(), vsb, e0[0], e0[1], njc, e0[2])
                pend = []
                pvs = pvs_box[0]
                # full-width zero matmuls close each pv accumulation group
                # (the trimmed final diagonal chunk only covers its tail cols)
                for h in range(HPC):
                    nc.tensor.matmul(
                        pvs[h][0 : DK + 1, 0:TW],
                        zer[:, 0 : DK + 1],
                        wsrc,
                        start=False,
                        stop=True,
                    )
                # drain pv banks: denom rows first (they gate the reciprocal
                # chain); a 16-descriptor DMA reshapes the denom row
                # [1, 4*512] into [16, 128] so the reciprocal runs on 16 lanes
                dns = rpool.tile([P, HPC * TW], F32, name="dns", tag="dns")
                dnr16 = rpool.tile([P, P], F32, name="dnr16", tag="dnr16")
                for h in range(HPC):
                    nc.vector.tensor_copy(
                        dns[DK : DK + 1, ds(TW * h, TW)], pvs[h][DK : DK + 1, :]
                    )
                nc.gpsimd.dma_start(
                    out=dnr16[0 : 4 * HPC, 0:P],
                    in_=dns[DK : DK + 1, :].rearrange("a (g c) -> a g c", c=P),
                )
                praws = []
                for h in range(HPC):
                    praw = rpool.tile([P, TW], BF16, name="praw", tag=f"praw{h}", bufs=2)
                    praws.append(praw)
                # praw h0/h1 copies fill the DVE while the gather DMA runs;
                # h2/h3 follow the reciprocal+cast so they don't delay it
                for h in (0, 1):
                    nc.vector.tensor_copy(praws[h][0:DK, :], pvs[h][0:DK, :])

                while fills:  # QKV for window w+1 must finish before its scores
                    fills.popleft()()
                if w + 2 <= W - 1:
                    fills = deque(qkv_fills(w + 2))

                def _norm_half(first, w=w, dnr16=dnr16, praws=praws, pvs_box=pvs_box):
                    # 16-lane reciprocal, bf16 cast, then a 16-descriptor DMA
                    # reshapes back to [4, 512] (row per head); K=4 selector
                    # matmuls broadcast each head's row across 64 partitions,
                    # rotating through the freed pv2/pv3 banks
                    if first:
                        rec16 = rpool.tile([P, P], F32, name="rec16", tag="rec16")
                        nc.vector.reciprocal(
                            out=rec16[0 : 4 * HPC, 0:P], in_=dnr16[0 : 4 * HPC, 0:P]
                        )
                        rec16b = rpool.tile([P, P], BF16, name="rec16b", tag="rec16b")
                        nc.vector.tensor_copy(
                            rec16b[0 : 4 * HPC, 0:P], rec16[0 : 4 * HPC, 0:P]
                        )
                        recb = rpool.tile([P, TW], BF16, name="recb", tag="recb")
                        nc.gpsimd.dma_start(
                            out=recb[0:HPC, :].rearrange("h (g c) -> h g c", c=P),
                            in_=rec16b[0 : 4 * HPC, 0:P],
                        )
                        for h in (2, 3):
                            nc.vector.tensor_copy(
                                praws[h][0:DK, :], pvs_box[0][h][0:DK, :]
                            )
                        _norm_half.recb = recb
                    recb = _norm_half.recb
                    for h in (0, 1) if first else (2, 3):
                        rt = psa.tile([P, TW], F32, tag=f"pv{h}", bufs=1, name="rt")
                        nc.tensor.matmul(
                            rt[0:DK, 0:TW],
                            sel[0:HPC, ds(DK * h, DK)],
                            recb[0:HPC, :],
                            start=True,
                            stop=True,
                        )
                        nc.vector.tensor_mul(
                            aI[0:DK, ds(RPW * w, RPW), ds(h, 1), :],
                            rt[0:DK, :].rearrange("p (r o t) -> p r o t", o=1, t=TT),
                            praws[h][0:DK, :].rearrange(
                                "p (r o t) -> p r o t", o=1, t=TT
                            ),
                        )
                    if not first:
                        # shifted copy (all heads at once): rows 64:128 at col
                        # c hold the value of col c+1; odd-t columns carry
                        # cross-block garbage but are never read
                        nc.gpsimd.dma_start(
                            out=att2a[DK : 2 * DK, ds(HPC * TW * w, HPC * TW - 1)],
                            in_=att2a[0:DK, ds(HPC * TW * w + 1, HPC * TW - 1)],
                        )

                def _projs(w=w):
                    # all 4 heads stacked into M=128: yp row r'*4+h is
                    # y row h*128 + 32w + r'. Both feature halves land in one
                    # [P, E] staging tile so the output DMA writes full 4KB
                    # rows, split across 4 queues.
                    ysb = ypool.tile([P, E], F32, name="ysb")
                    for fw in range(FW):
                        yp = psa.tile([P, TW], F32, tag=f"pv{fw}", bufs=1, name="yp")
                        for m in range(EC):
                            nc.tensor.matmul(
                                yp,
                                vM[:, ds(P * w, P), ds(2 * m, 1)],
                                wp_sb[:, m, ds(512 * fw, 512)],
                                start=(m == 0),
                                stop=False,
                            )
                        nc.tensor.matmul(
                            yp,
                            ones[0:1, 0:P],
                            bp_sb[0:1, ds(512 * fw, 512)],
                            start=False,
                            stop=True,
                        )
                        nc.vector.tensor_copy(ysb[:, ds(512 * fw, 512)], yp)
                    yv = y[:, :].rearrange("(h r) f -> r h f", r=RR)
                    yqs = [nc.sync, nc.scalar, nc.gpsimd]
                    for q in range(8):
                        yqs[q % 3].dma_start(
                            out=yv[ds(RPW * w + 4 * q, 4), :, :],
                            in_=ysb[ds(16 * q, 16), :],
                        )

                defw = deque(
                    [
                        lambda nh=_norm_half: nh(True),
                        lambda nh=_norm_half: nh(False),
                        _projs,
                    ]
                )

            # ---------------- tail: window 3 norm + projection ----------------
            # small warm chain keeps the PE clock ramped across the reciprocal
            # chain latency before the final rt/proj matmuls
            wt = psa.tile([P, 2 * TW], F32, tag="s", bufs=2, name="wt")
            NWARMT = 14
            for i in range(NWARMT):
                nc.tensor.matmul(
                    wt[0 : DK + 1, 0:TW],
                    zer[:, 0 : DK + 1],
                    wsrc,
                    start=(i == 0),
                    stop=(i == NWARMT - 1),
                )
            while defw:
                defw.popleft()()
    nc.compile()
    return nc


def _emit_pv_h(nc, pvs, vsb, ess, jc, njc, t0, h):
    """Single-head PV matmul for one jc (used by the per-head tail drain)."""
    p, sub = divmod(h, 2)
    nc.tensor.matmul(
        pvs[h][0 : DK + 1, ds(t0, TW - t0)],
        vsb[:, jc, ds((DK + 1) * h, DK + 1)],
        ess[p][:, ds(TW * sub + t0, TW - t0)],
        start=(jc == 0),
        stop=False,
        skip_group_check=(t0 > 0),
    )


def _emit_pv(nc, pvs, vsb, ess, jc, njc, t0=0):
    """P@V matmuls (M=65: V plus ones column -> denominator row 64) for one
    jc. On diagonal blocks (t0>0) the fully-masked leading columns skip."""
    for p in range(2):
        es = ess[p]
        for sub in range(2):
            h = 2 * p + sub
            nc.tensor.matmul(
                pvs[h][0 : DK + 1, ds(t0, TW - t0)],
                vsb[:, jc, ds((DK + 1) * h, DK + 1)],
                es[:, ds(TW * sub + t0, TW - t0)],
                start=(jc == 0),
                stop=False,
                skip_group_check=(t0 > 0),
            )


_CACHE = {}
LAST_RESULT = None


def _get_nc(T=2048):
    key = ("nc", T)
    if key not in _CACHE:
        _CACHE[key] = build_nc(T=T)
    return _CACHE[key]


def make_in_maps(x, w_qkv, w_proj, b_proj):
    B, T, _E = x.shape
    in_maps = []
    wpTh = np.ascontiguousarray(w_proj.T.astype(BF16NP))
    bph = np.ascontiguousarray(b_proj.reshape(1, E).astype(BF16NP))
    xTs = [np.ascontiguousarray(x[b].T.astype(BF16NP)) for b in range(B)]
    for c in range(8):
        b, g = divmod(c, 4)
        r0 = HPC * DK * g  # 256*g
        sl = slice(r0, r0 + HPC * DK)
        wqkvT = np.concatenate(
            [
                w_qkv[sl, :].T,
                w_qkv[E:][sl, :].T,
                w_qkv[2 * E :][sl, :].T,
            ],
            axis=1,
        )
        in_maps.append(
            {
                "xT": xTs[b],
                "wqkvT": np.ascontiguousarray(wqkvT.astype(BF16NP)),
                "wpT": wpTh,
                "bp": bph,
            }
        )
    return in_maps


def kernel(x, w_qkv, w_proj, b_proj):
    global LAST_RESULT
    from concourse.bass_utils import run_bass_kernel_spmd

    x = np.asarray(x, dtype=np.float32)
    w_qkv = np.asarray(w_qkv, dtype=np.float32)
    w_proj = np.asarray(w_proj, dtype=np.float32)
    b_proj = np.asarray(b_proj, dtype=np.float32)
    B, T, _E = x.shape

    nc = _get_nc(T=T)
    in_maps = make_in_maps(x, w_qkv, w_proj, b_proj)
    res = run_bass_kernel_spmd(nc, in_maps, core_ids=list(range(8)))
    LAST_RESULT = res

    out = np.empty((B, T, E), dtype=np.float32)
    rows = HPC * ((T * DK) // E)  # 512 rows per core
    for c in range(8):
        b, g = divmod(c, 4)
        out[b, rows * g : rows * (g + 1), :] = res.results[c]["y"]
    return out


# revision 11
# speedup vs baseline: 1.1783x; 1.0245x over previous
"""Causal attention (with faithful missing-head-transpose reshape bug) on 8 Trainium2 cores.

Problem: B=2, T=2048, E=1024, H=16, dk=64.
  qkv = x @ w_qkv.T ; q,k,v split; per-head causal softmax attention;
  out = att_out[B,H,T,dk].reshape(B,T,E)  (NO head transpose — faithful bug);
  y = out @ w_proj.T + b_proj

Because of the missing transpose, output rows y[b, 128h:128h+128, :] depend
ONLY on head h, so (batch x head-group) sharding over 8 cores needs no
collectives: core c handles batch c//4 and heads 4*(c%4)..4*(c%4)+3.

v2 schedule (single fully-interleaved pipeline; PE is the bottleneck engine
at ~130us of useful work, everything else hides under it):
  - input DMA is window-sliced and priority-ordered (wq, x-w0, wk, wv, x-w1,
    x-w2, wp, x-w3) so the first QKV matmuls start ~2us in; a PE warm chain
    runs during the DMA latency so the clock p-state is ramped before real work
  - QKV projection for window w+1 is emitted as fill closures pulled between
    attention jc-groups of window w — real work replaces v1's zero-matmul
    "HAM warmers"
  - scores are computed transposed (keys on partitions); exp tiles double as
    lhsT-ready P^T; diagonal blocks skip their fully-masked leading columns
    (N-trim) in scores/exp/PV and apply a single 128x128 triangle mask
  - V tiles are 128 wide per head (64 V cols, a ones column at 64+h, zeros
    elsewhere) so each head's softmax denominator lands on its own psum
    partition; the reciprocal then runs on [4,512] DVE lanes — no DMA spread
    round-trip and no 1-lane ops (DVE reciprocal is ~6.3 cycles/element)
  - the output projection for window w runs DURING window w+1 with all four
    heads stacked into M=128 (att2a is h-major so one lhsT AP spans heads),
    reusing freed pv PSUM banks; only window 3's norm+proj remain as tail
"""

import os
import sys
from collections import deque

import numpy as np

for _p in ("/opt/trn_rl_repo", "/root/.axon_site/_ro/trn_rl_repo"):
    if os.path.isdir(_p) and _p not in sys.path:
        sys.path.insert(0, _p)

import ml_dtypes  # noqa: E402

import concourse.bacc as bacc  # noqa: E402
import concourse.mybir as mybir  # noqa: E402
from concourse.bass import ds, ts  # noqa: E402
from concourse.tile import TileContext  # noqa: E402

F32 = mybir.dt.float32
BF16 = mybir.dt.bfloat16
AF = mybir.ActivationFunctionType
BF16NP = ml_dtypes.bfloat16

P = 128
E = 1024
DK = 64
HPC = 4  # heads per core
TW = 512  # i-window for scores / pv matmuls
EC = E // P  # 8 e-chunks
DC = (HPC * DK) // P  # 2 chunks of per-core qk features
FW = E // 512  # 2 output-feature windows


def build_nc(T=2048):
    W = T // TW  # 4 i-windows
    JPW = TW // P  # 4 j-chunks per window
    TC = T // P  # 16 t-chunks for V
    RR = (T * DK) // E  # 128 rows of R per head
    TT = E // DK  # 16 t-positions per R row
    RPW = RR // W  # 32 rows per head per window

    nc = bacc.Bacc("TRN2", target_bir_lowering=False, debug=False)
    xT = nc.declare_dram_parameter("xT", [E, T], BF16, isOutput=False)
    # q|k|v weight slices concatenated on the host: 3x fewer, 3x larger DMA
    # descriptors than three separate 512B-per-row loads
    wqkvT = nc.declare_dram_parameter("wqkvT", [E, 3 * HPC * DK], BF16, isOutput=False)
    wpT = nc.declare_dram_parameter("wpT", [E, E], BF16, isOutput=False)
    bp = nc.declare_dram_parameter("bp", [1, E], BF16, isOutput=False)
    y = nc.declare_dram_parameter("y", [HPC * RR, E], F32, isOutput=True)

    with nc.allow_low_precision(reason="bf16 matmuls; accumulation stays fp32 in PSUM"), TileContext(nc) as tc:
        with (
            tc.tile_pool(name="const", bufs=1) as const,
            tc.tile_pool(name="wts", bufs=1) as wts,
            tc.tile_pool(name="xin", bufs=1) as xpool,
            tc.tile_pool(name="qkvout", bufs=1) as qkv_pool,
            tc.tile_pool(name="att", bufs=1) as att_pool,
            tc.tile_pool(name="exps", bufs=15) as epool,
            tc.tile_pool(name="rec", bufs=2) as rpool,
            tc.tile_pool(name="yout", bufs=2) as ypool,
            tc.tile_pool(name="psa", bufs=1, space="PSUM") as psa,
        ):
            # ---------------- constants ----------------
            ones = const.tile([P, P], BF16)
            nc.vector.memset(ones, 1.0)
            zer = const.tile([P, P], BF16)
            nc.vector.memset(zer, 0.0)
            wsrc = const.tile([P, TW], BF16)
            nc.vector.memset(wsrc, 0.0)
            # lower-triangle-inclusive mask (keep i_local >= j_local)
            trimask = const.tile([P, P], BF16, name="trimask", tag="trimask")
            nc.vector.memset(trimask, 1.0)
            nc.gpsimd.affine_select(
                out=trimask,
                in_=trimask,
                pattern=[[1, P]],
                compare_op=mybir.AluOpType.is_ge,
                fill=0.0,
                base=0,
                channel_multiplier=-1,
            )
            bp_sb = const.tile([1, E], BF16)
            # head-selector for the reciprocal broadcast matmul: row j of
            # column block h is 1 iff j==h (K=4 matmul, base partition 0);
            # built with two affine_selects since memsets can't start at
            # partitions other than 0/32/64
            sel = const.tile([P, HPC * DK], BF16, name="sel", tag="sel")
            nc.vector.memset(sel[0:HPC, :], 1.0)
            nc.gpsimd.affine_select(
                out=sel[0:HPC, :],
                in_=sel[0:HPC, :],
                pattern=[[1, HPC * DK]],
                compare_op=mybir.AluOpType.is_ge,
                fill=0.0,
                base=0,
                channel_multiplier=-DK,
            )
            nc.gpsimd.affine_select(
                out=sel[0:HPC, :],
                in_=sel[0:HPC, :],
                pattern=[[-1, HPC * DK]],
                compare_op=mybir.AluOpType.is_ge,
                fill=0.0,
                base=DK - 1,
                channel_multiplier=DK,
            )

            wqkv_sb = wts.tile([P, EC, 3 * HPC * DK], BF16)
            wp_sb = wts.tile([P, EC, E], BF16)
            xp = xpool.tile([P, EC, T], BF16)

            qT = qkv_pool.tile([P, DC, T], BF16)
            kT = qkv_pool.tile([P, DC, T], BF16)
            vsb = qkv_pool.tile([P, TC, HPC * (DK + 1)], BF16)
            v4 = vsb.rearrange("p t (h c) -> p t h c", c=DK + 1)
            # ones column per head (PV psum row 64 = softmax denominator)
            nc.vector.memset(v4[:, :, :, DK : DK + 1], 1.0)

            # attention output, (r, h, t)-interleaved columns: col r*64+h*16+t;
            # rows 0:64 hold att[t], rows 64:128 hold att[t+1] (shifted copy;
            # only even-t columns of the shifted half are ever read). With this
            # order the projection's stacked-M index m = r'*4+h is a SINGLE
            # stride-16 AP dimension (BIR allows only one free dim on the
            # stationary matmul operand).
            att2a = att_pool.tile([P, HPC * T], BF16, name="att2a", tag="att2a")
            aI = att2a.rearrange("p (r h t) -> p r h t", h=HPC, t=TT)
            vM = att2a.rearrange("p (m s) -> p m s", s=TT)

            # dummy exp: pulls the ACT exp table load into the DMA wait
            es_d = epool.tile([P, 2 * TW], BF16, name="es_d")
            nc.scalar.activation(es_d[:, 0:8], zer[:, 0:8], AF.Exp, scale=1.0)

            # ---------------- input DMA schedule ----------------
            engs = [nc.sync, nc.gpsimd, nc.scalar]
            for e in range(EC):
                engs[e % 3].dma_start(out=wqkv_sb[:, e, :], in_=wqkvT[ts(e, P), :])
            for e in range(EC):
                engs[e % 3].dma_start(
                    out=xp[:, e, ds(0, TW)], in_=xT[ts(e, P), ds(0, TW)]
                )
            engs[0].dma_start(out=bp_sb, in_=bp[:, :])
            for wd in (1, 2):
                for e in range(EC):
                    engs[e % 3].dma_start(
                        out=xp[:, e, ds(TW * wd, TW)], in_=xT[ts(e, P), ds(TW * wd, TW)]
                    )
            for e in range(EC):
                engs[e % 3].dma_start(out=wp_sb[:, e, :], in_=wpT[ts(e, P), :])
            for e in range(EC):
                engs[e % 3].dma_start(
                    out=xp[:, e, ds(TW * 3, TW)], in_=xT[ts(e, P), ds(TW * 3, TW)]
                )

            # ---------------- PE warm chain during the DMA wait ----------------
            warm = psa.tile([P, 2 * TW], F32, tag="s", bufs=2, name="warm")
            NWARM0 = 10
            for i in range(NWARM0):
                nc.tensor.matmul(
                    warm[0 : DK + 1, 0:TW],
                    zer[:, 0 : DK + 1],
                    wsrc,
                    start=(i == 0),
                    stop=False,
                )

            # ---------------- QKV emission (window 0 eager, rest as fills) ----
            def mk_qk(dst, coff, wd, dc, warmers=0):
                def go():
                    ps = psa.tile([P, 2 * TW], F32, tag="s", bufs=2, name="ps_qk")
                    for e in range(EC):
                        nc.tensor.matmul(
                            ps[:, 0:TW],
                            wqkv_sb[:, e, ds(coff + P * dc, P)],
                            xp[:, e, ds(TW * wd, TW)],
                            start=(e == 0),
                            stop=(e == EC - 1),
                        )
                        for k in range(warmers):
                            nc.tensor.matmul(
                                warm[0 : DK + 1, 0:TW],
                                zer[:, 0 : DK + 1],
                                wsrc,
                                start=False,
                                stop=(e == EC - 1 and k == warmers - 1),
                            )
                    nc.vector.tensor_copy(dst[:, dc, ds(TW * wd, TW)], ps[:, 0:TW])

                return go

            def mk_v(t):
                def go():
                    ps = psa.tile([P, 2 * TW], F32, tag="s", bufs=2, name="ps_v")
                    for e in range(EC):
                        nc.tensor.matmul(
                            ps[:, 0 : HPC * DK],
                            xp[:, e, ts(t, P)],
                            wqkv_sb[:, e, ds(2 * HPC * DK, HPC * DK)],
                            start=(e == 0),
                            stop=(e == EC - 1),
                        )
                    nc.vector.tensor_copy(
                        v4[:, t, :, 0:DK],
                        ps[:, 0 : HPC * DK].rearrange("p (h d) -> p h d", d=DK),
                    )

                return go

            def qkv_fills(wd):
                out = [mk_qk(kT, HPC * DK, wd, dc) for dc in range(DC)]
                out += [mk_qk(qT, 0, wd, dc) for dc in range(DC)]
                out += [mk_v(t) for t in range(JPW * wd, JPW * (wd + 1))]
                return out

            # window 0, DMA-paced; warm matmuls interleave with the first chain
            mk_qk(qT, 0, 0, 0, warmers=2)()
            mk_qk(qT, 0, 0, 1)()
            mk_qk(kT, HPC * DK, 0, 0)()
            mk_qk(kT, HPC * DK, 0, 1)()
            for t in range(JPW):
                mk_v(t)()

            fills = deque(qkv_fills(1))

            # ---------------- attention + fused projection ----------------
            defw = deque()  # deferred norm/proj closures for window w-1
            for w in range(W):
                pvs_box = [None]

                def get_pvs(pvs_box=pvs_box):
                    if pvs_box[0] is None:
                        pvs_box[0] = [
                            psa.tile([P, TW], F32, tag=f"pv{h}", bufs=1, name=f"pv{h}")
                            for h in range(HPC)
                        ]
                    return pvs_box[0]

                njc = JPW * (w + 1)
                pend = []
                pop_from = 5 if w > 0 else 2
                for jc in range(njc):
                    qq = jc - JPW * w  # >=0 on causal-diagonal blocks
                    t0 = P * qq if qq > 0 else 0
                    ess = []
                    for p in range(2):
                        st = psa.tile([P, 2 * TW], F32, tag="s", bufs=2, name="st")
                        for sub in range(2):
                            nc.tensor.matmul(
                                st[:, ds(TW * sub + t0, TW - t0)],
                                kT[ds(DK * sub, DK), p, ts(jc, P)],
                                qT[ds(DK * sub, DK), p, ds(TW * w + t0, TW - t0)],
                                start=True,
                                stop=True,
                            )
                        es = epool.tile([P, 2 * TW], BF16, name="es")
                        if t0 == 0:
                            nc.scalar.activation(es, st, AF.Exp, scale=1.0 / 8.0)
                        else:
                            nc.scalar.activation(
                                es.rearrange("p (s n) -> p s n", s=2)[:, :, t0:TW],
                                st.rearrange("p (s n) -> p s n", s=2)[:, :, t0:TW],
                                AF.Exp,
                                scale=1.0 / 8.0,
                            )
                        if qq >= 0:
                            for sub in range(2):
                                nc.vector.tensor_mul(
                                    es[:, ds(TW * sub + t0, P)],
                                    es[:, ds(TW * sub + t0, P)],
                                    trimask,
                                )
                        ess.append(es)
                    pend.append((ess, jc, t0))

                    if jc in (2, 3, 4) and defw:
                        defw.popleft()()
                    if len(pend) > 2 and jc >= pop_from:
                        e0 = pend.pop(0)
                        _emit_pv(nc, get_pvs(), vsb, e0[0], e0[1], njc, e0[2])
                    for _ in range(2 if jc < 4 else 1):
                        if fills:
                            fills.popleft()()
                for e0 in pend:
                    _emit_pv(nc, get_pvs(), vsb, e0[0], e0[1], njc, e0[2])
                pend = []
                pvs = pvs_box[0]
                # full-width zero matmuls close each pv accumulation group
                # (the trimmed final diagonal chunk only covers its tail cols)
                for h in range(HPC):
                    nc.tensor.matmul(
                        pvs[h][0 : DK + 1, 0:TW],
                        zer[:, 0 : DK + 1],
                        wsrc,
                        start=False,
                        stop=True,
                    )
                # drain pv banks: denom rows first (they gate the reciprocal
                # chain); a 16-descriptor DMA reshapes the denom row
                # [1, 4*512] into [16, 128] so the reciprocal runs on 16 lanes
                dns = rpool.tile([P, HPC * TW], F32, name="dns", tag="dns")
                dnr16 = rpool.tile([P, P], F32, name="dnr16", tag="dnr16")
                for h in range(HPC):
                    nc.vector.tensor_copy(
                        dns[DK : DK + 1, ds(TW * h, TW)], pvs[h][DK : DK + 1, :]
                    )
                nc.gpsimd.dma_start(
                    out=dnr16[0 : 4 * HPC, 0:P],
                    in_=dns[DK : DK + 1, :].rearrange("a (g c) -> a g c", c=P),
                )
                praws = []
                for h in range(HPC):
                    praw = rpool.tile([P, TW], BF16, name="praw", tag=f"praw{h}", bufs=2)
                    praws.append(praw)
                # praw h0/h1 copies fill the DVE while the gather DMA runs;
                # h2/h3 follow the reciprocal+cast so they don't delay it
                for h in (0, 1):
                    nc.vector.tensor_copy(praws[h][0:DK, :], pvs[h][0:DK, :])

                while fills:  # QKV for window w+1 must finish before its scores
                    fills.popleft()()
                if w + 2 <= W - 1:
                    fills = deque(qkv_fills(w + 2))

                def _norm_half(first, w=w, dnr16=dnr16, praws=praws, pvs_box=pvs_box):
                    # 16-lane reciprocal, bf16 cast, then a 16-descriptor DMA
                    # reshapes back to [4, 512] (row per head); K=4 selector
                    # matmuls broadcast each head's row across 64 partitions,
                    # rotating through the freed pv2/pv3 banks
                    if first:
                        rec16 = rpool.tile([P, P], F32, name="rec16", tag="rec16")
                        nc.vector.reciprocal(
                            out=rec16[0 : 4 * HPC, 0:P], in_=dnr16[0 : 4 * HPC, 0:P]
                        )
                        rec16b = rpool.tile([P, P], BF16, name="rec16b", tag="rec16b")
                        nc.vector.tensor_copy(
                            rec16b[0 : 4 * HPC, 0:P], rec16[0 : 4 * HPC, 0:P]
                        )
                        recb = rpool.tile([P, TW], BF16, name="recb", tag="recb")
                        nc.gpsimd.dma_start(
                            out=recb[0:HPC, :].rearrange("h (g c) -> h g c", c=P),
                            in_=rec16b[0 : 4 * HPC, 0:P],
                        )
                        for h in (2, 3):
                            nc.vector.tensor_copy(
                                praws[h][0:DK, :], pvs_box[0][h][0:DK, :]
                            )
                        _norm_half.recb = recb
                    recb = _norm_half.recb
                    for h in (0, 1) if first else (2, 3):
                        rt = psa.tile([P, TW], F32, tag=f"pv{h}", bufs=1, name="rt")
                        nc.tensor.matmul(
                            rt[0:DK, 0:TW],
                            sel[0:HPC, ds(DK * h, DK)],
                            recb[0:HPC, :],
                            start=True,
                            stop=True,
                        )
                        nc.vector.tensor_mul(
                            aI[0:DK, ds(RPW * w, RPW), ds(h, 1), :],
                            rt[0:DK, :].rearrange("p (r o t) -> p r o t", o=1, t=TT),
                            praws[h][0:DK, :].rearrange(
                                "p (r o t) -> p r o t", o=1, t=TT
                            ),
                        )
                    if not first:
                        # shifted copy (all heads at once): rows 64:128 at col
                        # c hold the value of col c+1; odd-t columns carry
                        # cross-block garbage but are never read
                        nc.gpsimd.dma_start(
                            out=att2a[DK : 2 * DK, ds(HPC * TW * w, HPC * TW - 1)],
                            in_=att2a[0:DK, ds(HPC * TW * w + 1, HPC * TW - 1)],
                        )

                def _projs(w=w):
                    # all 4 heads stacked into M=128: yp row r'*4+h is
                    # y row h*128 + 32w + r'. Both feature halves land in one
                    # [P, E] staging tile so the output DMA writes full 4KB
                    # rows, split across 4 queues.
                    ysb = ypool.tile([P, E], F32, name="ysb")
                    for fw in range(FW):
                        yp = psa.tile([P, TW], F32, tag=f"pv{fw}", bufs=1, name="yp")
                        for m in range(EC):
                            nc.tensor.matmul(
                                yp,
                                vM[:, ds(P * w, P), ds(2 * m, 1)],
                                wp_sb[:, m, ds(512 * fw, 512)],
                                start=(m == 0),
                                stop=False,
                            )
                        nc.tensor.matmul(
                            yp,
                            ones[0:1, 0:P],
                            bp_sb[0:1, ds(512 * fw, 512)],
                            start=False,
                            stop=True,
                        )
                        nc.vector.tensor_copy(ysb[:, ds(512 * fw, 512)], yp)
                        yv = y[:, :].rearrange("(h r) f -> r h f", r=RR)
                        yqs = [nc.sync, nc.scalar, nc.gpsimd]
                        for q in range(3):
                            r0, r1 = 11 * q, min(32, 11 * q + 11)
                            yqs[q].dma_start(
                                out=yv[
                                    ds(RPW * w + r0, r1 - r0), :, ds(512 * fw, 512)
                                ],
                                in_=ysb[ds(4 * r0, 4 * (r1 - r0)), ds(512 * fw, 512)],
                            )

                defw = deque(
                    [
                        lambda nh=_norm_half: nh(True),
                        lambda nh=_norm_half: nh(False),
                        _projs,
                    ]
                )

            # ---------------- tail: window 3 norm + projection ----------------
            # small warm chain keeps the PE clock ramped across the reciprocal
            # chain latency before the final rt/proj matmuls
            wt = psa.tile([P, 2 * TW], F32, tag="s", bufs=2, name="wt")
            NWARMT = 32
            for i in range(NWARMT):
                nc.tensor.matmul(
                    wt[0 : DK + 1, 0:TW],
                    zer[:, 0 : DK + 1],
                    wsrc,
                    start=(i == 0),
                    stop=(i == NWARMT - 1),
                )
            while defw:
                defw.popleft()()
    nc.compile()
    return nc


def _emit_pv_h(nc, pvs, vsb, ess, jc, njc, t0, h):
    """Single-head PV matmul for one jc (used by the per-head tail drain)."""
    p, sub = divmod(h, 2)
    nc.tensor.matmul(
        pvs[h][0 : DK + 1, ds(t0, TW - t0)],
        vsb[:, jc, ds((DK + 1) * h, DK + 1)],
        ess[p][:, ds(TW * sub + t0, TW - t0)],
        start=(jc == 0),
        stop=False,
        skip_group_check=(t0 > 0),
    )


def _emit_pv(nc, pvs, vsb, ess, jc, njc, t0=0):
    """P@V matmuls (M=65: V plus ones column -> denominator row 64) for one
    jc. On diagonal blocks (t0>0) the fully-masked leading columns skip."""
    for p in range(2):
        es = ess[p]
        for sub in range(2):
            h = 2 * p + sub
            nc.tensor.matmul(
                pvs[h][0 : DK + 1, ds(t0, TW - t0)],
                vsb[:, jc, ds((DK + 1) * h, DK + 1)],
                es[:, ds(TW * sub + t0, TW - t0)],
                start=(jc == 0),
                stop=False,
                skip_group_check=(t0 > 0),
            )


_CACHE = {}
LAST_RESULT = None


def _get_nc(T=2048):
    key = ("nc", T)
    if key not in _CACHE:
        _CACHE[key] = build_nc(T=T)
    return _CACHE[key]


def make_in_maps(x, w_qkv, w_proj, b_proj):
    B, T, _E = x.shape
    in_maps = []
    wpTh = np.ascontiguousarray(w_proj.T.astype(BF16NP))
    bph = np.ascontiguousarray(b_proj.reshape(1, E).astype(BF16NP))
    xTs = [np.ascontiguousarray(x[b].T.astype(BF16NP)) for b in range(B)]
    for c in range(8):
        b, g = divmod(c, 4)
        r0 = HPC * DK * g  # 256*g
        sl = slice(r0, r0 + HPC * DK)
        wqkvT = np.concatenate(
            [
                w_qkv[sl, :].T,
                w_qkv[E:][sl, :].T,
                w_qkv[2 * E :][sl, :].T,
            ],
            axis=1,
        )
        in_maps.append(
            {
                "xT": xTs[b],
                "wqkvT": np.ascontiguousarray(wqkvT.astype(BF16NP)),
                "wpT": wpTh,
                "bp": bph,
            }
        )
    return in_maps


def kernel(x, w_qkv, w_proj, b_proj):
    global LAST_RESULT
    from concourse.bass_utils import run_bass_kernel_spmd

    x = np.asarray(x, dtype=np.float32)
    w_qkv = np.asarray(w_qkv, dtype=np.float32)
    w_proj = np.asarray(w_proj, dtype=np.float32)
    b_proj = np.asarray(b_proj, dtype=np.float32)
    B, T, _E = x.shape

    nc = _get_nc(T=T)
    in_maps = make_in_maps(x, w_qkv, w_proj, b_proj)
    res = run_bass_kernel_spmd(nc, in_maps, core_ids=list(range(8)))
    LAST_RESULT = res

    out = np.empty((B, T, E), dtype=np.float32)
    rows = HPC * ((T * DK) // E)  # 512 rows per core
    for c in range(8):
        b, g = divmod(c, 4)
        out[b, rows * g : rows * (g + 1), :] = res.results[c]["y"]
    return out


# revision 12
# speedup vs baseline: 1.1955x; 1.0146x over previous
"""Causal attention (with faithful missing-head-transpose reshape bug) on 8 Trainium2 cores.

Problem: B=2, T=2048, E=1024, H=16, dk=64.
  qkv = x @ w_qkv.T ; q,k,v split; per-head causal softmax attention;
  out = att_out[B,H,T,dk].reshape(B,T,E)  (NO head transpose — faithful bug);
  y = out @ w_proj.T + b_proj

Because of the missing transpose, output rows y[b, 128h:128h+128, :] depend
ONLY on head h, so (batch x head-group) sharding over 8 cores needs no
collectives: core c handles batch c//4 and heads 4*(c%4)..4*(c%4)+3.

v2 schedule (single fully-interleaved pipeline; PE is the bottleneck engine
at ~130us of useful work, everything else hides under it):
  - input DMA is window-sliced and priority-ordered (wq, x-w0, wk, wv, x-w1,
    x-w2, wp, x-w3) so the first QKV matmuls start ~2us in; a PE warm chain
    runs during the DMA latency so the clock p-state is ramped before real work
  - QKV projection for window w+1 is emitted as fill closures pulled between
    attention jc-groups of window w — real work replaces v1's zero-matmul
    "HAM warmers"
  - scores are computed transposed (keys on partitions); exp tiles double as
    lhsT-ready P^T; diagonal blocks skip their fully-masked leading columns
    (N-trim) in scores/exp/PV and apply a single 128x128 triangle mask
  - V tiles are 128 wide per head (64 V cols, a ones column at 64+h, zeros
    elsewhere) so each head's softmax denominator lands on its own psum
    partition; the reciprocal then runs on [4,512] DVE lanes — no DMA spread
    round-trip and no 1-lane ops (DVE reciprocal is ~6.3 cycles/element)
  - the output projection for window w runs DURING window w+1 with all four
    heads stacked into M=128 (att2a is h-major so one lhsT AP spans heads),
    reusing freed pv PSUM banks; only window 3's norm+proj remain as tail
"""

import os
import sys
from collections import deque

import numpy as np

for _p in ("/opt/trn_rl_repo", "/root/.axon_site/_ro/trn_rl_repo"):
    if os.path.isdir(_p) and _p not in sys.path:
        sys.path.insert(0, _p)

import ml_dtypes  # noqa: E402

import concourse.bacc as bacc  # noqa: E402
import concourse.mybir as mybir  # noqa: E402
from concourse.bass import ds, ts  # noqa: E402
from concourse.tile import TileContext  # noqa: E402

F32 = mybir.dt.float32
BF16 = mybir.dt.bfloat16
AF = mybir.ActivationFunctionType
BF16NP = ml_dtypes.bfloat16

P = 128
E = 1024
DK = 64
HPC = 4  # heads per core
TW = 512  # i-window for scores / pv matmuls
EC = E // P  # 8 e-chunks
DC = (HPC * DK) // P  # 2 chunks of per-core qk features
FW = E // 512  # 2 output-feature windows


def build_nc(T=2048):
    W = T // TW  # 4 i-windows
    JPW = TW // P  # 4 j-chunks per window
    TC = T // P  # 16 t-chunks for V
    RR = (T * DK) // E  # 128 rows of R per head
    TT = E // DK  # 16 t-positions per R row
    RPW = RR // W  # 32 rows per head per window

    nc = bacc.Bacc("TRN2", target_bir_lowering=False, debug=False)
    xT = nc.declare_dram_parameter("xT", [E, T], BF16, isOutput=False)
    # q|k|v weight slices concatenated on the host: 3x fewer, 3x larger DMA
    # descriptors than three separate 512B-per-row loads
    wqkvT = nc.declare_dram_parameter("wqkvT", [E, 3 * HPC * DK], BF16, isOutput=False)
    wpT = nc.declare_dram_parameter("wpT", [E, E], BF16, isOutput=False)
    bp = nc.declare_dram_parameter("bp", [1, E], BF16, isOutput=False)
    y = nc.declare_dram_parameter("y", [HPC * RR, E], F32, isOutput=True)

    with nc.allow_low_precision(reason="bf16 matmuls; accumulation stays fp32 in PSUM"), TileContext(nc) as tc:
        with (
            tc.tile_pool(name="const", bufs=1) as const,
            tc.tile_pool(name="wts", bufs=1) as wts,
            tc.tile_pool(name="xin", bufs=1) as xpool,
            tc.tile_pool(name="qkvout", bufs=1) as qkv_pool,
            tc.tile_pool(name="att", bufs=1) as att_pool,
            tc.tile_pool(name="exps", bufs=15) as epool,
            tc.tile_pool(name="rec", bufs=2) as rpool,
            tc.tile_pool(name="yout", bufs=2) as ypool,
            tc.tile_pool(name="psa", bufs=1, space="PSUM") as psa,
        ):
            # ---------------- constants ----------------
            ones = const.tile([P, P], BF16)
            nc.vector.memset(ones, 1.0)
            zer = const.tile([P, P], BF16)
            nc.vector.memset(zer, 0.0)
            wsrc = const.tile([P, TW], BF16)
            nc.vector.memset(wsrc, 0.0)
            # lower-triangle-inclusive mask (keep i_local >= j_local)
            trimask = const.tile([P, P], BF16, name="trimask", tag="trimask")
            nc.vector.memset(trimask, 1.0)
            nc.gpsimd.affine_select(
                out=trimask,
                in_=trimask,
                pattern=[[1, P]],
                compare_op=mybir.AluOpType.is_ge,
                fill=0.0,
                base=0,
                channel_multiplier=-1,
            )
            bp_sb = const.tile([1, E], BF16)
            # head-selector for the reciprocal broadcast matmul: row j of
            # column block h is 1 iff j==h (K=4 matmul, base partition 0);
            # built with two affine_selects since memsets can't start at
            # partitions other than 0/32/64
            sel = const.tile([P, HPC * DK], BF16, name="sel", tag="sel")
            nc.vector.memset(sel[0:HPC, :], 1.0)
            nc.gpsimd.affine_select(
                out=sel[0:HPC, :],
                in_=sel[0:HPC, :],
                pattern=[[1, HPC * DK]],
                compare_op=mybir.AluOpType.is_ge,
                fill=0.0,
                base=0,
                channel_multiplier=-DK,
            )
            nc.gpsimd.affine_select(
                out=sel[0:HPC, :],
                in_=sel[0:HPC, :],
                pattern=[[-1, HPC * DK]],
                compare_op=mybir.AluOpType.is_ge,
                fill=0.0,
                base=DK - 1,
                channel_multiplier=DK,
            )

            wqkv_sb = wts.tile([P, EC, 3 * HPC * DK], BF16)
            wp_sb = wts.tile([P, EC, E], BF16)
            xp = xpool.tile([P, EC, T], BF16)

            qT = qkv_pool.tile([P, DC, T], BF16)
            kT = qkv_pool.tile([P, DC, T], BF16)
            vsb = qkv_pool.tile([P, TC, HPC * (DK + 1)], BF16)
            v4 = vsb.rearrange("p t (h c) -> p t h c", c=DK + 1)
            # ones column per head (PV psum row 64 = softmax denominator)
            nc.vector.memset(v4[:, :, :, DK : DK + 1], 1.0)

            # attention output, (r, h, t)-interleaved columns: col r*64+h*16+t;
            # rows 0:64 hold att[t], rows 64:128 hold att[t+1] (shifted copy;
            # only even-t columns of the shifted half are ever read). With this
            # order the projection's stacked-M index m = r'*4+h is a SINGLE
            # stride-16 AP dimension (BIR allows only one free dim on the
            # stationary matmul operand).
            att2a = att_pool.tile([P, HPC * T], BF16, name="att2a", tag="att2a")
            aI = att2a.rearrange("p (r h t) -> p r h t", h=HPC, t=TT)
            vM = att2a.rearrange("p (m s) -> p m s", s=TT)

            # dummy exp: pulls the ACT exp table load into the DMA wait
            es_d = epool.tile([P, 2 * TW], BF16, name="es_d")
            nc.scalar.activation(es_d[:, 0:8], zer[:, 0:8], AF.Exp, scale=1.0)

            # ---------------- input DMA schedule ----------------
            engs = [nc.sync, nc.gpsimd, nc.scalar]
            for e in range(EC):
                engs[e % 3].dma_start(out=wqkv_sb[:, e, :], in_=wqkvT[ts(e, P), :])
            for e in range(EC):
                engs[e % 3].dma_start(
                    out=xp[:, e, ds(0, TW)], in_=xT[ts(e, P), ds(0, TW)]
                )
            engs[0].dma_start(out=bp_sb, in_=bp[:, :])
            for wd in (1, 2):
                for e in range(EC):
                    engs[e % 3].dma_start(
                        out=xp[:, e, ds(TW * wd, TW)], in_=xT[ts(e, P), ds(TW * wd, TW)]
                    )
            for e in range(EC):
                engs[e % 3].dma_start(out=wp_sb[:, e, :], in_=wpT[ts(e, P), :])
            for e in range(EC):
                engs[e % 3].dma_start(
                    out=xp[:, e, ds(TW * 3, TW)], in_=xT[ts(e, P), ds(TW * 3, TW)]
                )

            # ---------------- PE warm chain during the DMA wait ----------------
            warm = psa.tile([P, 2 * TW], F32, tag="s", bufs=2, name="warm")
            NWARM0 = 16
            for i in range(NWARM0):
                nc.tensor.matmul(
                    warm[0 : DK + 1, 0:TW],
                    zer[:, 0 : DK + 1],
                    wsrc,
                    start=(i == 0),
                    stop=(i == NWARM0 - 1),
                )

            # ---------------- QKV emission (window 0 eager, rest as fills) ----
            def mk_qk(dst, coff, wd, dc, warmers=0):
                def go():
                    ps = psa.tile([P, 2 * TW], F32, tag="s", bufs=2, name="ps_qk")
                    for e in range(EC):
                        nc.tensor.matmul(
                            ps[:, 0:TW],
                            wqkv_sb[:, e, ds(coff + P * dc, P)],
                            xp[:, e, ds(TW * wd, TW)],
                            start=(e == 0),
                            stop=(e == EC - 1),
                        )
                        for k in range(warmers):
                            nc.tensor.matmul(
                                warm[0 : DK + 1, 0:TW],
                                zer[:, 0 : DK + 1],
                                wsrc,
                                start=False,
                                stop=(e == EC - 1 and k == warmers - 1),
                            )
                    nc.vector.tensor_copy(dst[:, dc, ds(TW * wd, TW)], ps[:, 0:TW])

                return go

            def mk_v(t):
                def go():
                    ps = psa.tile([P, 2 * TW], F32, tag="s", bufs=2, name="ps_v")
                    for e in range(EC):
                        nc.tensor.matmul(
                            ps[:, 0 : HPC * DK],
                            xp[:, e, ts(t, P)],
                            wqkv_sb[:, e, ds(2 * HPC * DK, HPC * DK)],
                            start=(e == 0),
                            stop=(e == EC - 1),
                        )
                    nc.vector.tensor_copy(
                        v4[:, t, :, 0:DK],
                        ps[:, 0 : HPC * DK].rearrange("p (h d) -> p h d", d=DK),
                    )

                return go

            def qkv_fills(wd):
                out = [mk_qk(kT, HPC * DK, wd, dc) for dc in range(DC)]
                out += [mk_qk(qT, 0, wd, dc) for dc in range(DC)]
                out += [mk_v(t) for t in range(JPW * wd, JPW * (wd + 1))]
                return out

            # window 0, DMA-paced: the dc0/dc1 chains are interleaved per
            # e-chunk so each arriving x slice unlocks two matmuls instead of
            # the second chain head-of-line blocking on the first's last chunk
            def qk_pair(dst, coff):
                pa = psa.tile([P, 2 * TW], F32, tag="s", bufs=2, name="ps_a")
                pb = psa.tile([P, 2 * TW], F32, tag="s", bufs=2, name="ps_b")
                for e in range(EC):
                    for dc, ps in ((0, pa), (1, pb)):
                        nc.tensor.matmul(
                            ps[:, 0:TW],
                            wqkv_sb[:, e, ds(coff + P * dc, P)],
                            xp[:, e, ds(0, TW)],
                            start=(e == 0),
                            stop=(e == EC - 1),
                        )
                nc.vector.tensor_copy(dst[:, 0, ds(0, TW)], pa[:, 0:TW])
                nc.vector.tensor_copy(dst[:, 1, ds(0, TW)], pb[:, 0:TW])

            qk_pair(qT, 0)
            qk_pair(kT, HPC * DK)
            for t in range(JPW):
                mk_v(t)()

            fills = deque(qkv_fills(1))

            # ---------------- attention + fused projection ----------------
            defw = deque()  # deferred norm/proj closures for window w-1
            for w in range(W):
                pvs_box = [None]

                def get_pvs(pvs_box=pvs_box):
                    if pvs_box[0] is None:
                        pvs_box[0] = [
                            psa.tile([P, TW], F32, tag=f"pv{h}", bufs=1, name=f"pv{h}")
                            for h in range(HPC)
                        ]
                    return pvs_box[0]

                njc = JPW * (w + 1)
                pend = []
                pop_from = 5 if w > 0 else 2
                for jc in range(njc):
                    qq = jc - JPW * w  # >=0 on causal-diagonal blocks
                    t0 = P * qq if qq > 0 else 0
                    ess = []
                    for p in range(2):
                        st = psa.tile([P, 2 * TW], F32, tag="s", bufs=2, name="st")
                        for sub in range(2):
                            nc.tensor.matmul(
                                st[:, ds(TW * sub + t0, TW - t0)],
                                kT[ds(DK * sub, DK), p, ts(jc, P)],
                                qT[ds(DK * sub, DK), p, ds(TW * w + t0, TW - t0)],
                                start=True,
                                stop=True,
                            )
                        es = epool.tile([P, 2 * TW], BF16, name="es")
                        if t0 == 0:
                            nc.scalar.activation(es, st, AF.Exp, scale=1.0 / 8.0)
                        else:
                            nc.scalar.activation(
                                es.rearrange("p (s n) -> p s n", s=2)[:, :, t0:TW],
                                st.rearrange("p (s n) -> p s n", s=2)[:, :, t0:TW],
                                AF.Exp,
                                scale=1.0 / 8.0,
                            )
                        if qq >= 0:
                            for sub in range(2):
                                nc.vector.tensor_mul(
                                    es[:, ds(TW * sub + t0, P)],
                                    es[:, ds(TW * sub + t0, P)],
                                    trimask,
                                )
                        ess.append(es)
                    pend.append((ess, jc, t0))

                    if jc in (2, 3, 4) and defw:
                        defw.popleft()()
                    if len(pend) > 2 and jc >= pop_from:
                        e0 = pend.pop(0)
                        _emit_pv(nc, get_pvs(), vsb, e0[0], e0[1], njc, e0[2])
                    for _ in range(2 if jc < 4 else 1):
                        if fills:
                            fills.popleft()()
                for e0 in pend:
                    _emit_pv(nc, get_pvs(), vsb, e0[0], e0[1], njc, e0[2])
                pend = []
                pvs = pvs_box[0]
                # full-width zero matmuls close each pv accumulation group
                # (the trimmed final diagonal chunk only covers its tail cols)
                for h in range(HPC):
                    nc.tensor.matmul(
                        pvs[h][0 : DK + 1, 0:TW],
                        zer[:, 0 : DK + 1],
                        wsrc,
                        start=False,
                        stop=True,
                    )
                # drain pv banks: denom rows first (they gate the reciprocal
                # chain); a 16-descriptor DMA reshapes the denom row
                # [1, 4*512] into [16, 128] so the reciprocal runs on 16 lanes
                dns = rpool.tile([P, HPC * TW], F32, name="dns", tag="dns")
                dnr16 = rpool.tile([P, P], F32, name="dnr16", tag="dnr16")
                for h in range(HPC):
                    nc.vector.tensor_copy(
                        dns[DK : DK + 1, ds(TW * h, TW)], pvs[h][DK : DK + 1, :]
                    )
                nc.gpsimd.dma_start(
                    out=dnr16[0 : 4 * HPC, 0:P],
                    in_=dns[DK : DK + 1, :].rearrange("a (g c) -> a g c", c=P),
                )
                praws = []
                for h in range(HPC):
                    praw = rpool.tile([P, TW], BF16, name="praw", tag=f"praw{h}", bufs=2)
                    praws.append(praw)
                # praw h0/h1 copies fill the DVE while the gather DMA runs;
                # h2/h3 follow the reciprocal+cast so they don't delay it
                for h in (0, 1):
                    nc.vector.tensor_copy(praws[h][0:DK, :], pvs[h][0:DK, :])

                while fills:  # QKV for window w+1 must finish before its scores
                    fills.popleft()()
                if w + 2 <= W - 1:
                    fills = deque(qkv_fills(w + 2))

                def _norm_half(first, w=w, dnr16=dnr16, praws=praws, pvs_box=pvs_box):
                    # 16-lane reciprocal, bf16 cast, then a 16-descriptor DMA
                    # reshapes back to [4, 512] (row per head); K=4 selector
                    # matmuls broadcast each head's row across 64 partitions,
                    # rotating through the freed pv2/pv3 banks
                    if first:
                        rec16 = rpool.tile([P, P], F32, name="rec16", tag="rec16")
                        nc.vector.reciprocal(
                            out=rec16[0 : 4 * HPC, 0:P], in_=dnr16[0 : 4 * HPC, 0:P]
                        )
                        rec16b = rpool.tile([P, P], BF16, name="rec16b", tag="rec16b")
                        nc.vector.tensor_copy(
                            rec16b[0 : 4 * HPC, 0:P], rec16[0 : 4 * HPC, 0:P]
                        )
                        recb = rpool.tile([P, TW], BF16, name="recb", tag="recb")
                        nc.gpsimd.dma_start(
                            out=recb[0:HPC, :].rearrange("h (g c) -> h g c", c=P),
                            in_=rec16b[0 : 4 * HPC, 0:P],
                        )
                        for h in (2, 3):
                            nc.vector.tensor_copy(
                                praws[h][0:DK, :], pvs_box[0][h][0:DK, :]
                            )
                        _norm_half.recb = recb
                    recb = _norm_half.recb
                    for h in (0, 1) if first else (2, 3):
                        rt = psa.tile([P, TW], F32, tag=f"pv{h}", bufs=1, name="rt")
                        nc.tensor.matmul(
                            rt[0:DK, 0:TW],
                            sel[0:HPC, ds(DK * h, DK)],
                            recb[0:HPC, :],
                            start=True,
                            stop=True,
                        )
                        nc.vector.tensor_mul(
                            aI[0:DK, ds(RPW * w, RPW), ds(h, 1), :],
                            rt[0:DK, :].rearrange("p (r o t) -> p r o t", o=1, t=TT),
                            praws[h][0:DK, :].rearrange(
                                "p (r o t) -> p r o t", o=1, t=TT
                            ),
                        )
                    if not first:
                        # shifted copy (all heads at once): rows 64:128 at col
                        # c hold the value of col c+1; odd-t columns carry
                        # cross-block garbage but are never read
                        nc.gpsimd.dma_start(
                            out=att2a[DK : 2 * DK, ds(HPC * TW * w, HPC * TW - 1)],
                            in_=att2a[0:DK, ds(HPC * TW * w + 1, HPC * TW - 1)],
                        )

                def _projs(w=w):
                    # all 4 heads stacked into M=128: yp row r'*4+h is
                    # y row h*128 + 32w + r'. Both feature halves land in one
                    # [P, E] staging tile so the output DMA writes full 4KB
                    # rows, split across 4 queues.
                    ysb = ypool.tile([P, E], F32, name="ysb")
                    for fw in range(FW):
                        yp = psa.tile([P, TW], F32, tag=f"pv{fw}", bufs=1, name="yp")
                        for m in range(EC):
                            nc.tensor.matmul(
                                yp,
                                vM[:, ds(P * w, P), ds(2 * m, 1)],
                                wp_sb[:, m, ds(512 * fw, 512)],
                                start=(m == 0),
                                stop=False,
                            )
                        nc.tensor.matmul(
                            yp,
                            ones[0:1, 0:P],
                            bp_sb[0:1, ds(512 * fw, 512)],
                            start=False,
                            stop=True,
                        )
                        nc.vector.tensor_copy(ysb[:, ds(512 * fw, 512)], yp)
                        yv = y[:, :].rearrange("(h r) f -> r h f", r=RR)
                        yqs = [nc.sync, nc.scalar, nc.gpsimd]
                        for q in range(3):
                            r0, r1 = 11 * q, min(32, 11 * q + 11)
                            yqs[q].dma_start(
                                out=yv[
                                    ds(RPW * w + r0, r1 - r0), :, ds(512 * fw, 512)
                                ],
                                in_=ysb[ds(4 * r0, 4 * (r1 - r0)), ds(512 * fw, 512)],
                            )

                defw = deque(
                    [
                        lambda nh=_norm_half: nh(True),
                        lambda nh=_norm_half: nh(False),
                        _projs,
                    ]
                )

            # ---------------- tail: window 3 norm + projection ----------------
            # small warm chain keeps the PE clock ramped across the reciprocal
            # chain latency before the final rt/proj matmuls
            wt = psa.tile([P, 2 * TW], F32, tag="s", bufs=2, name="wt")
            NWARMT = 32
            for i in range(NWARMT):
                nc.tensor.matmul(
                    wt[0 : DK + 1, 0:TW],
                    zer[:, 0 : DK + 1],
                    wsrc,
                    start=(i == 0),
                    stop=(i == NWARMT - 1),
                )
            while defw:
                defw.popleft()()
    nc.compile()
    return nc


def _emit_pv_h(nc, pvs, vsb, ess, jc, njc, t0, h):
    """Single-head PV matmul for one jc (used by the per-head tail drain)."""
    p, sub = divmod(h, 2)
    nc.tensor.matmul(
        pvs[h][0 : DK + 1, ds(t0, TW - t0)],
        vsb[:, jc, ds((DK + 1) * h, DK + 1)],
        ess[p][:, ds(TW * sub + t0, TW - t0)],
        start=(jc == 0),
        stop=False,
        skip_group_check=(t0 > 0),
    )


def _emit_pv(nc, pvs, vsb, ess, jc, njc, t0=0):
    """P@V matmuls (M=65: V plus ones column -> denominator row 64) for one
    jc. On diagonal blocks (t0>0) the fully-masked leading columns skip."""
    for p in range(2):
        es = ess[p]
        for sub in range(2):
            h = 2 * p + sub
            nc.tensor.matmul(
                pvs[h][0 : DK + 1, ds(t0, TW - t0)],
                vsb[:, jc, ds((DK + 1) * h, DK + 1)],
                es[:, ds(TW * sub + t0, TW - t0)],
                start=(jc == 0),
                stop=False,
                skip_group_check=(t0 > 0),
            )


_CACHE = {}
LAST_RESULT = None


def _get_nc(T=2048):
    key = ("nc", T)
    if key not in _CACHE:
        _CACHE[key] = build_nc(T=T)
    return _CACHE[key]


def make_in_maps(x, w_qkv, w_proj, b_proj):
    B, T, _E = x.shape
    in_maps = []
    wpTh = np.ascontiguousarray(w_proj.T.astype(BF16NP))
    bph = np.ascontiguousarray(b_proj.reshape(1, E).astype(BF16NP))
    xTs = [np.ascontiguousarray(x[b].T.astype(BF16NP)) for b in range(B)]
    for c in range(8):
        b, g = divmod(c, 4)
        r0 = HPC * DK * g  # 256*g
        sl = slice(r0, r0 + HPC * DK)
        wqkvT = np.concatenate(
            [
                w_qkv[sl, :].T,
                w_qkv[E:][sl, :].T,
                w_qkv[2 * E :][sl, :].T,
            ],
            axis=1,
        )
        in_maps.append(
            {
                "xT": xTs[b],
                "wqkvT": np.ascontiguousarray(wqkvT.astype(BF16NP)),
                "wpT": wpTh,
                "bp": bph,
            }
        )
    return in_maps


def kernel(x, w_qkv, w_proj, b_proj):
    global LAST_RESULT
    from concourse.bass_utils import run_bass_kernel_spmd

    x = np.asarray(x, dtype=np.float32)
    w_qkv = np.asarray(w_qkv, dtype=np.float32)
    w_proj = np.asarray(w_proj, dtype=np.float32)
    b_proj = np.asarray(b_proj, dtype=np.float32)
    B, T, _E = x.shape

    nc = _get_nc(T=T)
    in_maps = make_in_maps(x, w_qkv, w_proj, b_proj)
    res = run_bass_kernel_spmd(nc, in_maps, core_ids=list(range(8)))
    LAST_RESULT = res

    out = np.empty((B, T, E), dtype=np.float32)
    rows = HPC * ((T * DK) // E)  # 512 rows per core
    for c in range(8):
        b, g = divmod(c, 4)
        out[b, rows * g : rows * (g + 1), :] = res.results[c]["y"]
    return out
